# revision 43
# baseline (speedup 1.0000x reference)
"""Trainium2 Bass kernel for the 20-layer dilated-causal-conv audio model.

Formulation:
- Only the last 128 output timesteps are needed -> per-layer suffix pyramid.
  Layer i only computes timesteps in blocks [TB[i+1], 512) of 16 steps each.
- Channels are tiny (8), so convs run on the TensorEngine as block-Toeplitz
  matmuls: partition dim = 16 timesteps x 8 channels = 128; each conv tap is a
  host-built 128x128 stationary matrix; taps accumulate in PSUM.
- The per-layer control (1x1 conv on ctrl) is one extra matmul with
  contraction 16; all per-channel constants (conv bias, ctrl bias, folded
  io_b drift) ride the ReLU activation's per-partition bias.
- Residual 1x1 (io_w) is a block-diagonal matmul; the residual add runs on
  the VectorEngine. Each layer's last-8-block h slice is stashed (GPSIMD
  SBUF copy) and the mixer runs as a short layer-major PSUM accumulation per
  batch pair at the tail.
- Data parallel over batch: 32 batches -> 8 cores x 4; per core, 2 chains of
  2 batches pipelined software-style (each unit's mixer/residual trails its
  conv by one unit) so ReLU/residual-add latency hides under the other
  chain's conv work.
- All 128-partition weights are host-packed partition-major into one DRAM
  tensor and staged as a handful of group DMAs ordered by first-use, so
  layer 0 starts ~2.5us in and later groups land before their layers.
"""

import numpy as np

import concourse.bass as bass
import concourse.mybir as mybir
import concourse.tile as tile
import concourse.bass_utils as _bass_utils
from concourse.bass_utils import run_bass_kernel_spmd

# NOTE: fp16/bf16 matmuls lower to Ldweights+Matmult pairs; the separate
# stationary load costs ~96ns of real PE time per matmul (unmodeled by the
# cost sim, and walrus's ldw-opt pass is both disabled and broken in this
# build). float32r self-loads its weights, so it pays no such tax; with
# bpc_c=2 nearly all matmuls have free size >= 256 where f32r runs at the
# same 1 cycle/row as fp16.

# ---------------------------------------------------------------- constants
DIL = [1, 2, 4, 8, 16, 32, 64, 128, 256, 512] * 2
NL = 20          # layers
CH = 8           # channels
BLK = 16         # timesteps per block
NB = 512         # blocks in T=8192
T = 8192
B = 32           # total batch
NCORES = 8
BPC = B // NCORES  # batch per core
NCHAINS = 1      # single chain: chunk-level wavefront hides latency with
                 # ~half the instruction count of 2 batch chains (measured
                 # faster on HW; per-instruction overheads dominate)

# matmul dtype: float16. Measured on this hardware (microbench, free=512):
# f32r matmuls run ~1.4 cyc/col (not the cost model's 1.0 -- the inline
# stationary self-load steals moving-stream bandwidth) and 1.64 cyc/col at
# free=256; fp16/bf16 run 1.0 cyc/col at any free size with only ~28 ns per
# Ldweights (FWL), and halve weight-DMA bytes and DVE element cost. fp16's
# 10-bit mantissa keeps end-to-end rel err at 1.4e-3 (bf16: 1.2e-2).
DT = mybir.dt.float16
NPDT = np.float16

# dev-only ablation switches for cost-model attribution (subset of
# {"act", "dve", "mm", "wdma", "mix"}); empty for the real kernel
ABLATE = set()
MIXPSUM = True   # accumulate the mixer in one PSUM group instead of DVE adds
WDT = None       # weight dtype override (None -> DT)
WBUFS = 1        # weight pool bufs
PCBUFS = 4       # conv PSUM pool bufs
PIOBUFS = 2      # residual PSUM pool bufs (pc4+pio2+pmx2 = 8 banks)
POOL_RELU = set()  # GPSIMD cannot access PSUM (walrus birverifier) - keep empty
PIPELAG_MAX = 3  # max conv groups between a unit's conv (A) and its B;
                 # overlapping previous-layer B's are force-flushed earlier
MERGE_AT = 14    # from this layer on, both chains fuse into one bpc=4 unit
                 # (vacuous at NCHAINS=1; must stay < COMPOSE_FROM)
MIXFILL = 10     # lo-mixer matmuls interleaved per merged layer (stall fill)
CHUNK_MIN = {}   # layer -> minimum chunk count (deeper intra-layer pipeline)
DVE_RELU = ()    # tried range(6,16): DVE serializes relu with the residual
                 # adds and loses 1.3us -- keep everything on Act
                 # free size 4*ext stays >= 256 (f32r 1 cyc/row, no 4x penalty)
INPUTS_FIRST = True  # issue audio/ctrl DMAs before the weight stream
COMPOSE_FROM = 15  # layers >= this use residual composition: conv_i reads
                   # x_{i-1} via the normal taps plus h_{i-1} via
                   # iow-composed taps, so only relu (not iow+residual-add)
                   # gates the PE between consecutive merged layers
MIX_INLINE = True  # emit each mixer matmul right after its h stash instead
                   # of batching via MIXFILL/tail

# block-start table: TB[i] = first block of x~_i ; TB[NL] = first output block.
# Extents forced even: f32r matmuls need an even innermost free count.
TB = [0] * (NL + 1)
TB[NL] = NB - 8
for i in range(NL - 1, -1, -1):
    TB[i] = TB[i + 1] - max(1, (2 * DIL[i]) // BLK)
    if DT == mybir.dt.float32r and (NB - TB[i]) % 2:
        TB[i] -= 1

# per-layer tap metadata: list of (block_offset, tap_array_index)
_TAP_OFFSETS = []
_NT = 0
for _i in range(NL):
    d = DIL[_i]
    offs = [0, d // BLK, 2 * (d // BLK)] if d >= BLK else [0, 1]
    _TAP_OFFSETS.append([(o, _NT + j) for j, o in enumerate(offs)])
    _NT += len(offs)

# packed weight tensor (partition-major [128, _WF]): per layer, taps (m*128),
# iow (128, i<NL-1), mixw (16). Grouped into _NWG staged DMAs so layer 0's
# weights land in the first (small) transfer and compute starts immediately.
_TAP_F = [0] * NL
_IOW_F = [0] * (NL - 1)
_MIX_F = [0] * NL
_f = 0
for _i in range(NL):
    _TAP_F[_i] = _f
    _f += len(_TAP_OFFSETS[_i]) * 128
    if _i < NL - 1:
        _IOW_F[_i] = _f
        _f += 128
    _MIX_F[_i] = _f
    _f += 16
# residual-composed taps (layers >= COMPOSE_FROM): conv_i also gets
# iow_{i-1}-composed stationaries applied to h_{i-1}; appended at the end of
# the pack so they ride the last weight group.
_COMPOSE = list(range(max(COMPOSE_FROM, 1), NL))
_COMPOSE_SET = set(_COMPOSE)
_CTAP_F = {}
for _i in _COMPOSE:
    _CTAP_F[_i] = _f
    _f += len(_TAP_OFFSETS[_i]) * 128
_WF = _f
_WG_LAYERS = [0, 1, 3, 6, 10, 15, NL]   # group g covers layers [b[g], b[g+1])
_NWG = len(_WG_LAYERS) - 1
_WG_LO = [_TAP_F[_WG_LAYERS[g]] for g in range(_NWG)] + [_WF]
_LAYER_WG = [0] * NL
for _g in range(_NWG):
    for _i in range(_WG_LAYERS[_g], _WG_LAYERS[_g + 1]):
        _LAYER_WG[_i] = _g


# --------------------------------------------- redundant-Ldweights removal
def _dedup_ldweights(nc):
    """fp16 matmuls lower to Ldweights+Matmult pairs, and Tile's legalization
    reloads the stationary for every matmul (walrus ldw-opt is disabled and
    broken in this build). The PE's weight registers persist across matmuls,
    so a Ldweights whose weights AP is identical to the previous one on the
    same engine (with no intervening load and no sync obligations) is a
    ~96ns no-op: drop it. Emission orders same-stationary matmuls
    consecutively (tap-major across chains) to maximize hits."""

    def sig(inst):
        ap = inst.ins[0]
        try:
            t = ap.tensor_name if hasattr(ap, "tensor_name") else str(getattr(ap, "memory_location", ""))
        except Exception:
            t = ""
        return repr((t, str(ap)))

    removed = 0
    for f in nc.m.functions:
        for bb in f.blocks:
            last = None
            keep = []
            for inst in bb.instructions:
                if inst.engine != mybir.EngineType.PE:
                    keep.append(inst)
                    continue
                op = getattr(inst, "opcode", "")
                if op == "Ldweights":
                    si = inst.sync_info
                    clean = si is None or (not si.on_wait and not si.on_update)
                    s = sig(inst)
                    if clean and last == s:
                        removed += 1
                        continue
                    last = s
                elif op != "Matmult":
                    # any other PE instruction: weight state unknown
                    last = None
                keep.append(inst)
            bb.instructions[:] = keep
    return removed


# ------------------------------------------------- workaround: 1-wait limit
def _split_multi_waits(nc):
    """This walrus build allows only one sem wait per TPB instruction, but
    Tile's kernel-tail drain carries several. Move extras onto preceding
    same-engine nops (in-order execution keeps the gating semantics)."""
    tpb = {
        mybir.EngineType.SP,
        mybir.EngineType.PE,
        mybir.EngineType.DVE,
        mybir.EngineType.Activation,
        mybir.EngineType.Pool,
    }
    for f in nc.m.functions:
        for bb in f.blocks:
            new_list = []
            changed = False
            for inst in bb.instructions:
                si = inst.sync_info
                if si is not None and si.on_wait and len(si.on_wait) > 1 and inst.engine in tpb:
                    waits = list(si.on_wait)
                    for j, w in enumerate(waits[:-1]):
                        nop = mybir.InstNoOp(name=f"{inst.name}-ws{j}", ins=[], outs=[])
                        nop.engine = inst.engine
                        nop.sync_info = mybir.SyncInfo(on_wait=[w], on_update=[])
                        new_list.append(nop)
                    si.on_wait = waits[-1:]
                    changed = True
                new_list.append(inst)
            if changed:
                bb.instructions[:] = new_list


# ------------------------------------------------------------- host arrays
def _build_host_arrays(inputs):
    c_w0 = np.asarray(inputs["c_w0"], np.float32)    # [3,1,8]
    c_ws = np.asarray(inputs["c_ws"], np.float32)    # [19,3,8,8]
    c_b = np.asarray(inputs["c_b"], np.float32)      # [20,8]
    ctrl_w = np.asarray(inputs["ctrl_w"], np.float32)  # [20,1,1]
    ctrl_b = np.asarray(inputs["ctrl_b"], np.float32)  # [20,1]
    io_w = np.asarray(inputs["io_w"], np.float32)    # [19,8,8]
    io_b = np.asarray(inputs["io_b"], np.float32)    # [19,8]
    mix_w = np.asarray(inputs["mix_w"], np.float32)  # [160,1]

    tapw = np.zeros((_NT, 128, 128), np.float32)
    iow = np.zeros((NL - 1, 128, 128), np.float32)
    # auxw[NL] is the audio channel-broadcast matrix for the layer-0 residual
    auxw = np.zeros((NL + 1, 16, 128), np.float32)
    mixw = np.zeros((NL, 128, 16), np.float32)
    biases = np.zeros((128, NL), np.float32)
    for t in range(BLK):
        auxw[NL, t, t * 8 : t * 8 + 8] = 1.0

    const_i = np.zeros(CH, np.float32)
    for i in range(NL):
        w = c_w0 if i == 0 else c_ws[i - 1]          # [3, cin, 8]
        cin = w.shape[1]
        d = DIL[i]
        wD = [w[2], w[1], w[0]]                      # wD[l] multiplies x[t - l*d]
        bias = c_b[i] + ctrl_b[i][0]
        if cin == CH:
            bias = bias + np.einsum("kco,c->o", w, const_i)
        biases[:, i] = np.tile(bias, BLK)

        # layer 0 (cin=1) reads the 16-partition audio tile: row index = ti.
        # layers >0 read the 128-partition x~ tile: row index = ti*8 + ci.
        def rows(ti):
            return slice(ti, ti + 1) if cin == 1 else slice(ti * 8, ti * 8 + cin)

        if d >= BLK:
            for l, (_, idx) in enumerate(_TAP_OFFSETS[i]):
                W = tapw[idx]
                for t in range(BLK):
                    W[rows(t), t * 8 : t * 8 + 8] = wD[l][:cin]
        else:
            Wc = tapw[_TAP_OFFSETS[i][0][1]]
            Wp = tapw[_TAP_OFFSETS[i][1][1]]
            for to in range(BLK):
                for l in range(3):
                    ti = to - l * d
                    if ti >= 0:
                        Wc[rows(ti), to * 8 : to * 8 + 8] += wD[l][:cin]
                    else:
                        Wp[rows(ti + BLK), to * 8 : to * 8 + 8] += wD[l][:cin]

        for t in range(BLK):
            auxw[i, t, t * 8 : t * 8 + 8] = ctrl_w[i][0, 0]
            mixw[i, t * 8 : t * 8 + 8, t] = mix_w[i * 8 : i * 8 + 8, 0]
        if i < NL - 1:
            for t in range(BLK):
                iow[i, t * 8 : t * 8 + 8, t * 8 : t * 8 + 8] = io_w[i]
            const_i = const_i + io_b[i]

    # pack all 128-partition weights into one partition-major tensor so each
    # group DMA is 128 long contiguous runs (no sub-512B descriptor penalty)
    wpack = np.zeros((128, _WF), np.float32)
    for i in range(NL):
        for j, (_, idx) in enumerate(_TAP_OFFSETS[i]):
            wpack[:, _TAP_F[i] + j * 128 : _TAP_F[i] + (j + 1) * 128] = tapw[idx]
        if i < NL - 1:
            wpack[:, _IOW_F[i] : _IOW_F[i] + 128] = iow[i]
        wpack[:, _MIX_F[i] : _MIX_F[i] + 16] = mixw[i]
    # iow-composed taps: conv_i(x_i) = taps_i(x_{i-1}) + (iow_{i-1}@taps_i)(h_{i-1})
    for i in _COMPOSE:
        for j, (_, idx) in enumerate(_TAP_OFFSETS[i]):
            wpack[:, _CTAP_F[i] + j * 128 : _CTAP_F[i] + (j + 1) * 128] = (
                iow[i - 1] @ tapw[idx]
            )
    return dict(
        wpack=wpack.astype(NPDT),
        auxw=np.ascontiguousarray(auxw.transpose(1, 0, 2)).astype(NPDT),
        biases=biases,
    )


# ----------------------------------------------------------- device program
_NC_CACHE = {}


def _chunks(out_b):
    """Right-aligned chunks over output blocks [out_b, NB): list of (lo, w),
    left to right; the rightmost chunk always covers the final 8 blocks."""
    out = []
    hi = NB
    while hi > out_b:
        lo = max(out_b, hi - CHUNK)
        out.append((lo, hi - lo))
        hi = lo
    return out[::-1]


def _build_nc(loop_k=None):
    """loop_k: dev-only probe mode — wrap the whole body in For_i(0, loop_k)
    so marginal per-iteration wall time on HW isolates kernel exec from the
    ~100ms dispatch floor."""
    nc = bass.Bass()
    f32 = mybir.dt.float32
    bpc_c = BPC // NCHAINS          # batch elements per chain
    chunkb = 512 // bpc_c           # output blocks per chunk (N <= 512)
    # composed layers need i-1 merged (single h tile) and single-chunk
    for _ci in _COMPOSE_SET:
        assert _ci > MERGE_AT, (_ci, MERGE_AT)
        assert NB - TB[_ci + 1] <= 512 // BPC, _ci

    nblk0 = NB - TB[0]
    nblk1 = NB - TB[1]
    # audio/ctrl arrive host-blocked as [16=t-in-block, BPC, nblk]
    audio_h = nc.dram_tensor("audio", [BLK, BPC, nblk0], DT, kind="ExternalInput")
    ctrl_h = nc.dram_tensor("ctrl", [BLK, BPC, nblk1], DT, kind="ExternalInput")
    wpack_h = nc.dram_tensor("wpack", [128, _WF], WDT or DT, kind="ExternalInput")
    auxw_h = nc.dram_tensor("auxw", [16, NL + 1, 128], WDT or DT, kind="ExternalInput")
    biases_h = nc.dram_tensor("biases", [128, NL], f32, kind="ExternalInput")
    out_h = nc.dram_tensor("out", [BPC, 128], f32, kind="ExternalOutput")

    import contextlib

    inline_k = 1
    if isinstance(loop_k, tuple):  # (outer For_i count, inline copies per pass)
        loop_k, inline_k = loop_k
    elif loop_k and loop_k < 0:    # negative: inline replication (no back-edge)
        inline_k, loop_k = -loop_k, None

    with tile.TileContext(nc) as tc:
        with (
            tc.For_i(0, loop_k, 1) if loop_k else contextlib.nullcontext(),
            tc.tile_pool(name="w", bufs=WBUFS) as wpool,
            tc.tile_pool(name="xs", bufs=1) as xpool,
            tc.tile_pool(name="h", bufs=6) as hpool,
            tc.tile_pool(name="pc", bufs=PCBUFS, space="PSUM") as pcpool,
            tc.tile_pool(name="pio", bufs=PIOBUFS, space="PSUM") as piopool,
            tc.tile_pool(name="pm", bufs=1, space="PSUM") as pmpool,
        ):
            for rep in range(inline_k):
                # ---- DMA plan: 8 triggers total (HWDGE issue is ~600ns of
                # engine-queue time each). Inputs + first weight group go
                # first; weight groups stage so layer i's weights land before
                # its compute. Spread across SP/Pool/Act queues.
                wdt = WDT or DT
                audio_t = xpool.tile([16, BPC, nblk0], DT, tag="x0", name="audio_t")
                ctrl_t = wpool.tile([16, BPC, nblk1], DT, name="ctrl_t")
                auxw_t = wpool.tile([16, NL + 1, 128], wdt)
                bias_t = wpool.tile([128, NL], f32)
                wgts = [
                    wpool.tile(
                        [128, _WG_LO[g + 1] - _WG_LO[g]],
                        wdt,
                        tag=f"wg{g}",
                        name=f"wg{g}",
                    )
                    for g in range(_NWG)
                ]
                # HWDGE queues are SP and Act only (SWDGE/Pool DMA cannot be
                # codegenned inside For_i). Ordered by first use: the very
                # first matmul is layer 0's ctrl matmul (ctrl_t + auxw_t),
                # then its taps (wg0 + audio_t), then the first ReLU (bias).
                # Act takes the two small tensors ahead of its Relu-table
                # warm-up, which overlaps the DMA prologue.
                nc.sync.dma_start(out=ctrl_t[:], in_=ctrl_h[:])
                nc.scalar.dma_start(out=auxw_t[:], in_=auxw_h[:])
                nc.sync.dma_start(out=wgts[0][:], in_=wpack_h[:, _WG_LO[0] : _WG_LO[1]])
                nc.scalar.dma_start(out=bias_t[:], in_=biases_h[:])
                nc.sync.dma_start(out=audio_t[:], in_=audio_h[:])
                for g in range(1, _NWG):
                    nc.sync.dma_start(
                        out=wgts[g][:], in_=wpack_h[:, _WG_LO[g] : _WG_LO[g + 1]]
                    )

                def tap_ap(i, j, kp):
                    g = _LAYER_WG[i]
                    a = _TAP_F[i] - _WG_LO[g] + j * 128
                    return wgts[g][:kp, a : a + 128]

                def iow_ap(i):
                    g = _LAYER_WG[i]
                    a = _IOW_F[i] - _WG_LO[g]
                    return wgts[g][:, a : a + 128]

                def ctap_ap(i, j):
                    g = _NWG - 1
                    a = _CTAP_F[i] - _WG_LO[g] + j * 128
                    return wgts[g][:, a : a + 128]

                def mix_ap(i):
                    g = _LAYER_WG[i]
                    a = _MIX_F[i] - _WG_LO[g]
                    return wgts[g][:, a : a + 16]

                zeros_t = None
                if DVE_RELU:
                    zeros_t = wpool.tile(
                        [128, 512], DT, name="zeros", tag="zeros"
                    )
                    nc.vector.memset(zeros_t, 0.0)

                # warm the Act engine's Relu table during the DMA prologue so
                # the first real ReLU doesn't eat the 1.3us table load
                warm_t = wpool.tile([128, 2], f32, name="warm", tag="warm")
                nc.vector.memset(warm_t, 0.0)
                nc.scalar.activation(
                    out=warm_t[:],
                    in_=warm_t[:],
                    func=mybir.ActivationFunctionType.Relu,
                )

                # x_ts[c]: current x~ tile of chain c ([128, bpc_c, nblk] or the
                # shared 16-partition audio tile at layer 0).
                #
                # Skewed schedule: at step s, chain c works on layer s-c, so
                # the PE stream mixes long early layers with short late ones.
                # Within a step, two phases: (A) every active chain's conv
                # (+ReLU on Act/Pool), then (B) every active chain's
                # mixer/residual. PE then always has several independent conv
                # groups queued ahead of any h- or x-dependent matmul: ReLU /
                # residual-add latency never stalls the PE (each PE gap also
                # resets its p-state ramp, halving the clock).
                x_ts = [audio_t] * NCHAINS
                hs = {}
                xns = {}
                xreg = {}  # (layer, chain-key) -> input x tile for that layer
                chunks_of = {}  # layer -> [(lo, w)] right-to-left
                pcs_of = {}  # (layer, chain, lo) -> prefetched conv PSUM

                def emitCtrl(i, c, lo, w):
                    # ctrl matmul + PSUM allocation, hoisted ahead of the
                    # whole layer: depends only on the prologue's ctrl_t, so
                    # these bridge the PE gap while the previous layer's
                    # relu/residual add (which gate the taps) are in flight
                    bw = bpc_c if i < MERGE_AT else BPC
                    b0 = c * bw
                    pc = pcpool.tile([128, bw, 512 // bw], f32, name="pc")
                    a = lo - TB[1]
                    nc.tensor.matmul(
                        pc[:, :, :w],
                        auxw_t[:, i, :],
                        ctrl_t[:, b0 : b0 + bw, a : a + w],
                        start=True,
                        stop=False,
                    )
                    pcs_of[(i, c, lo)] = pc

                def emitA(i, c, lo, w):  # ---- conv taps + relu of chunk
                    kp = 16 if i == 0 else 128
                    bw = bpc_c if i < MERGE_AT else BPC
                    b0 = c * bw
                    if i == 0:
                        x_t = audio_t
                    elif i in _COMPOSE_SET:
                        x_t = None  # composed: reads x_{i-1}/h_{i-1} instead
                    else:
                        x_t = xreg[(i, 0 if i >= MERGE_AT else c)]
                    xb = slice(b0, b0 + bw) if i == 0 else slice(0, bw)
                    pc = pcs_of.pop((i, c, lo))
                    nt = len(_TAP_OFFSETS[i])
                    if i in _COMPOSE_SET:
                        # residual composition: conv_i = taps_i(x_{i-1}) +
                        # (iow_{i-1} @ taps_i)(h_{i-1}). x_ts[0] still holds
                        # x_{i-1} because B(i-1) is pending; flush it here so
                        # its iow matmul lands between this ctrl matmul and
                        # the h-taps (everything after relu_{i-1} on the PE),
                        # and its residual add overlaps the h-taps.
                        # capture h_{i-1} tiles before the flush pops them;
                        # composed h-taps may span several h chunks
                        hmm = []
                        for j, (off, idx) in enumerate(_TAP_OFFSETS[i]):
                            lo_r, hi_r = lo - off, lo + w - off
                            for lo2, w2 in chunks_of[i - 1]:
                                s = max(lo_r, lo2)
                                e = min(hi_r, lo2 + w2)
                                if s < e:
                                    hmm.append(
                                        (j, s + off - lo, e - s,
                                         hs[(i - 1, 0, lo2)], s - lo2)
                                    )
                        x_prev = xreg[(i - 1, 0)] if i > 1 else audio_t
                        # x-taps first: they depend on add(i-2), long done, so
                        # they cover relu(i-1)'s latency on the PE; then B(i-1)
                        # (iow+mix, gated by relu(i-1)), then the h-taps
                        for j, (off, idx) in enumerate(_TAP_OFFSETS[i]):
                            a = lo - off - TB[i - 1]
                            nc.tensor.matmul(
                                pc[:, :, :w],
                                tap_ap(i, j, kp),
                                x_prev[:, 0:bw, a : a + w],
                                start=False,
                                stop=False,
                            )
                        while pending:
                            emitB(*pending.pop(0))
                        for m, (j, oa, mw, h_tile, ha) in enumerate(hmm):
                            nc.tensor.matmul(
                                pc[:, :, oa : oa + mw],
                                ctap_ap(i, j),
                                h_tile[:, 0:bw, ha : ha + mw],
                                start=False,
                                stop=(m == len(hmm) - 1),
                            )
                    else:
                        for j, (off, idx) in enumerate(_TAP_OFFSETS[i]):
                            a = lo - off - TB[i]
                            nc.tensor.matmul(
                                pc[:, :, :w],
                                tap_ap(i, j, kp),
                                x_t[:, xb, a : a + w],
                                start=False,
                                stop=(j == nt - 1),
                            )
                    h = hpool.tile([128, bw, 512 // bw], DT, name="h")
                    hs[(i, c, lo)] = h
                    if (c == 1 or NCHAINS == 1) and i in DVE_RELU:
                        # relu(pc + bias) on DVE: (pc add bias) max 0 --
                        # overlaps with chain-0's ReLU on Act
                        nc.vector.scalar_tensor_tensor(
                            out=h[:, :, :w],
                            in0=pc[:, :, :w],
                            scalar=bias_t[:, i : i + 1],
                            in1=zeros_t[:, : bw * w].rearrange(
                                "p (b q) -> p b q", b=bw
                            ),
                            op0=mybir.AluOpType.add,
                            op1=mybir.AluOpType.max,
                        )
                    else:
                        nc.scalar.activation(
                            out=h[:, :, :w],
                            in_=pc[:, :, :w],
                            func=mybir.ActivationFunctionType.Relu,
                            bias=bias_t[:, i : i + 1],
                            scale=1.0,
                        )

                def emitB(i, c, lo, w):  # ---- mixer + residual of chunk
                    out_b = TB[i + 1]
                    bw = bpc_c if i < MERGE_AT else BPC
                    b0 = c * bw
                    if i == 0:
                        x_t = audio_t
                    else:
                        x_t = xreg[(i, 0 if i >= MERGE_AT else c)]
                    h = hs.pop((i, c, lo))
                    if lo + w >= NB:
                        # one mixer matmul straight off the h tile's last 8
                        # blocks (all batches), accumulated layer-major into
                        # the shared pmx PSUM group
                        r = NB - 8 - lo
                        nc.tensor.matmul(
                            pmx_t[:, b0 : b0 + bw, :],
                            mix_ap(i),
                            h[:, :, r : r + 8],
                            start=(i == 0),
                            stop=(i == NL - 1),
                            skip_group_check=True,
                        )
                    if i == NL - 1:
                        return
                    # next layer's batch width decides the x_next tile shape
                    nbw = bpc_c if i + 1 < MERGE_AT else BPC
                    pio = piopool.tile([128, bw, 512 // bw], f32, name="pio")
                    if i == 0:
                        # audio-broadcast first: it depends only on the
                        # prologue's audio_t, so it runs while this unit's
                        # ReLU (which gates the iow matmul) is in flight
                        a = lo - TB[0]
                        nc.tensor.matmul(
                            pio[:, :, :w],
                            auxw_t[:, NL, :],
                            audio_t[:, b0 : b0 + bw, a : a + w],
                            start=True,
                            stop=False,
                        )
                        nc.tensor.matmul(
                            pio[:, :, :w],
                            iow_ap(i),
                            h[:, :, :w],
                            start=False,
                            stop=True,
                        )
                    else:
                        nc.tensor.matmul(
                            pio[:, :, :w],
                            iow_ap(i),
                            h[:, :, :w],
                            start=True,
                            stop=True,
                        )
                    # merged layers share one x_next tile across chains
                    xkey = (i, 0 if nbw == BPC else c)
                    if xkey in xns:
                        x_next = xns[xkey]
                    else:
                        x_next = xpool.tile(
                            [128, nbw, NB - out_b], DT,
                            tag=f"x{xkey[1]}_{i + 1}", name=f"x{xkey[1]}_{i + 1}",
                        )
                        xns[xkey] = x_next
                        xreg[(i + 1, xkey[1])] = x_next
                    xo = b0 if nbw == BPC and bw < BPC else 0
                    if i == 0:
                        nc.vector.tensor_copy(
                            out=x_next[:, xo : xo + bw, lo - out_b : lo - out_b + w],
                            in_=pio[:, :, :w],
                        )
                    else:
                        nc.vector.tensor_add(
                            out=x_next[:, xo : xo + bw, lo - out_b : lo - out_b + w],
                            in0=x_t[:, 0:bw, lo - TB[i] : lo - TB[i] + w],
                            in1=pio[:, :, :w],
                        )
                    if lo == out_b:  # leftmost chunk emitted last
                        x_ts[c if nbw != BPC else 0] = x_next
                        if nbw != BPC or (nbw == BPC and bw == BPC):
                            xns.pop(xkey, None)
                        elif c == NCHAINS - 1:
                            xns.pop(xkey, None)

                # software-pipelined emission over (layer, chain, chunk)
                # units: B of each unit trails its A by PIPELAG conv groups,
                # so PE never waits on relu latency and the residual add
                # lands before the next layer's taps need it. Chunks
                # right-to-left so the mixer-bearing chunk is ready first.
                units = []
                for i in range(NL):
                    nch = NCHAINS if i < MERGE_AT else 1
                    cb = chunkb if i < MERGE_AT else 512 // BPC
                    ext = NB - TB[i + 1]
                    if ext > cb or CHUNK_MIN.get(i, 1) > 1:
                        # balanced even split: all chunks keep free size
                        # >= 256 (f32r fast mode) instead of 256 + sliver
                        k = max(-(-ext // cb), CHUNK_MIN.get(i, 1))
                        cb = (-(-ext // k) + 1) // 2 * 2
                    chunks_of[i] = []
                    for c in range(nch):
                        hi = NB
                        while hi > TB[i + 1]:
                            lo = max(TB[i + 1], hi - cb)
                            units.append((i, c, lo, hi - lo))
                            if c == 0:
                                chunks_of[i].append((lo, hi - lo))
                            hi = lo
                pmx_t = pmpool.tile([16, BPC, 8], f32, name="pmx", tag="pmx")
                pending = []

                def flushB(v):
                    # emit a pending B after recursively emitting the older
                    # B's whose x writes its residual add reads
                    if v not in pending:
                        return
                    for v2 in [
                        p
                        for p in pending
                        if p[0] == v[0] - 1
                        and p[2] < v[2] + v[3]
                        and p[2] + p[3] > v[2]
                    ]:
                        flushB(v2)
                    pending.remove(v)
                    emitB(*v)

                for u in units:
                    i, c, lo, w = u
                    emitCtrl(*u)
                    if i in _COMPOSE_SET:
                        # composed layers flush B(i-1) inside emitA after
                        # their ctrl matmul
                        pass
                    elif NCHAINS == 1 and i > 0:
                        # wavefront: this chunk's taps read x_i over
                        # [lo - maxoff, lo + w); only the previous layer's
                        # B chunks overlapping that range must precede it.
                        # Left chunks of earlier layers keep flowing behind,
                        # so layer boundaries stop being global barriers.
                        maxoff = _TAP_OFFSETS[i][-1][0]
                        for v in [
                            p
                            for p in pending
                            if p[0] == i - 1
                            and p[2] < lo + w
                            and p[2] + p[3] > lo - maxoff
                        ]:
                            flushB(v)
                    elif u[0] >= MERGE_AT:
                        while pending:
                            emitB(*pending.pop(0))
                    else:
                        while any(
                            p[0] == u[0] - 1 and p[1] == u[1] for p in pending
                        ):
                            emitB(*pending.pop(0))
                    emitA(*u)
                    pending.append(u)
                    while len(pending) > PIPELAG_MAX:
                        emitB(*pending.pop(0))
                for u in pending:
                    emitB(*u)

                # tail: pmx PSUM -> SBUF -> one DRAM store
                out_t = wpool.tile([16, BPC, 8], f32, name="out_t", tag="out_t")
                nc.vector.tensor_copy(out=out_t[:], in_=pmx_t[:])
                dst = bass.AP(
                    tensor=out_h,
                    offset=0,
                    ap=[[1, BLK], [128, BPC], [BLK, 8]],
                )
                nc.sync.dma_start(out=dst, in_=out_t[:])

    _dedup_ldweights(nc)
    _split_multi_waits(nc)
    return nc


def _get_nc():
    if "nc" not in _NC_CACHE:
        _NC_CACHE["nc"] = _build_nc()
    return _NC_CACHE["nc"]


# ------------------------------------------------------------------- public
def _block(sig, b0):
    """[b, T] -> [16, b, nblk] suffix-block layout starting at block b0."""
    nblk = NB - b0
    v = sig[:, b0 * BLK :].reshape(sig.shape[0], nblk, BLK)
    return np.ascontiguousarray(v.transpose(2, 0, 1)).astype(NPDT)


def _make_in_maps(inputs):
    host = _build_host_arrays(inputs)
    audio = np.asarray(inputs["audio"], np.float32)[:, :, 0]
    ctrl = np.asarray(inputs["ctrl"], np.float32)[:, :, 0]

    in_maps = []
    for c in range(NCORES):
        sl = slice(c * BPC, (c + 1) * BPC)
        in_maps.append(
            {
                "audio": _block(audio[sl], TB[0]),
                "ctrl": _block(ctrl[sl], TB[1]),
                "wpack": host["wpack"],
                "auxw": host["auxw"],
                "biases": host["biases"],
            }
        )
    return in_maps


def kernel(**inputs) -> np.ndarray:
    nc = _get_nc()
    mix_b = float(np.asarray(inputs["mix_b"], np.float32)[0])
    in_maps = _make_in_maps(inputs)
    res = run_bass_kernel_spmd(nc, in_maps, core_ids=list(range(NCORES)))
    out = np.concatenate([res.results[c]["out"] for c in range(NCORES)], axis=0)
    return (out + mix_b).astype(np.float32)

